# revision 1
# baseline (speedup 1.0000x reference)
"""Llama4 MoE experts + shared LoRA, expert-parallel on 8 TRN2 NeuronCores.

Per-core (expert e): x[1024,1024] @ W_gu[1024,4096] (+ rank-8 LoRA) -> SwiGLU
-> h[1024,2048] @ W_dn[2048,1024] (+ rank-8 LoRA) -> out[1024,1024].

Layout strategy: keep the intermediate transposed. gate_up^T tiles
[128 F-part, 512 T-free] come straight out of PSUM with H contracted on the
partition dim; SwiGLU runs on the transposed tiles, producing hiddenT with I on
partitions -- exactly the contraction layout the down matmul needs, whose
output is then the natural [T, H]. Only x (4 MiB) is PE-transposed on chip.

All matmuls run in float32r (fp32 with 11-bit RNE mantissa, 1 cycle/row for
free-dim >= 256 -- bf16 speed at ~1.5e-4 relative error).
"""
import sys

sys.path.insert(0, "/opt/trn_rl_repo")

import numpy as np

import concourse.bacc as bacc
import concourse.bass as bass
import concourse.mybir as mybir
import concourse.tile as tile
from concourse.bass_utils import run_bass_kernel_spmd
from concourse.masks import make_identity

E = 8           # experts == cores
T = 1024        # tokens per expert
H = 1024        # hidden
I = 2048        # intermediate
F2 = 2 * I      # gate+up
R = 8           # lora rank
SCALING = 2.0   # lora_alpha / rank
P = 128         # partitions
NFREE = 512     # moving free-dim per matmul (one PSUM bank of fp32)
KH = H // P     # 8 k-tiles over H
KI = I // P     # 16 k-tiles over I
NT = T // NFREE     # 2 T-chunks
NH = H // NFREE     # 2 H-chunks
NFP = I // P        # 16 F-pair tiles (gate i pairs with up i+16)

F32 = mybir.dt.float32
F32R = mybir.dt.float32r


def build_kernel():
    nc = bacc.Bacc("TRN2", target_bir_lowering=False, debug=False)

    x_d = nc.dram_tensor("x", [T, H], F32, kind="ExternalInput")
    wgu_d = nc.dram_tensor("w_gu", [H, F2], F32R, kind="ExternalInput")
    wdn_d = nc.dram_tensor("w_dn", [I, H], F32R, kind="ExternalInput")
    agu_d = nc.dram_tensor("a_gu", [R, H], F32, kind="ExternalInput")
    bgu_d = nc.dram_tensor("b_gu", [F2, R], F32, kind="ExternalInput")
    adn_d = nc.dram_tensor("a_dn", [R, I], F32, kind="ExternalInput")
    bdn_d = nc.dram_tensor("b_dn", [H, R], F32, kind="ExternalInput")
    out_d = nc.dram_tensor("out", [T, H], F32, kind="ExternalOutput")

    with tile.TileContext(nc) as tc:
        with (
            tc.tile_pool(name="const", bufs=1) as const_pool,
            tc.tile_pool(name="xT", bufs=1) as xT_pool,
            tc.tile_pool(name="hT", bufs=1) as hT_pool,
            tc.tile_pool(name="smalls", bufs=1) as small_pool,
            tc.tile_pool(name="xnat", bufs=2) as xnat_pool,
            tc.tile_pool(name="wgu", bufs=4) as wgu_pool,
            tc.tile_pool(name="wdn", bufs=4) as wdn_pool,
            tc.tile_pool(name="silu", bufs=3) as silu_pool,
            tc.tile_pool(name="outs", bufs=3) as out_pool,
            tc.tile_pool(name="ps_tr", bufs=2, space="PSUM") as ps_tr,
            tc.tile_pool(name="ps_mm", bufs=6, space="PSUM") as ps_mm,
        ):
            ident = const_pool.tile([P, P], F32)
            make_identity(nc, ident[:])

            # ---- phase A: transpose x into xT[k] = [128 H, 1024 T] (f32r) ----
            xT = [xT_pool.tile([P, T], F32R, tag=f"xT{k}", name=f"xT{k}") for k in range(KH)]
            for b in range(T // P):
                xb = xnat_pool.tile([P, H], F32)
                nc.sync.dma_start(xb[:], x_d[P * b:P * (b + 1), :])
                for k in range(KH):
                    ps = ps_tr.tile([P, P], F32, tag="tr")
                    nc.tensor.transpose(ps[:], xb[:, P * k:P * (k + 1)], ident[:])
                    nc.vector.tensor_copy(xT[k][:, P * b:P * (b + 1)], ps[:])

            # ---- small lora tensors: transpose to matmul layouts (f32r) ----
            # A_guT[k]: [128 H, 8 R]
            agu_nat = small_pool.tile([R, H], F32, tag="agu_nat")
            nc.sync.dma_start(agu_nat[:], agu_d[:])
            aguT = [small_pool.tile([P, R], F32R, tag=f"aguT{k}", name=f"aguT{k}") for k in range(KH)]
            for k in range(KH):
                ps = ps_tr.tile([P, R], F32, tag="tr")
                nc.tensor.transpose(ps[:], agu_nat[:, P * k:P * (k + 1)],
                                    ident[:R, :R])
                nc.vector.tensor_copy(aguT[k][:], ps[:])
            # A_dnT[k]: [128 I, 8 R]
            adn_nat = small_pool.tile([R, I], F32, tag="adn_nat")
            nc.sync.dma_start(adn_nat[:], adn_d[:])
            adnT = [small_pool.tile([P, R], F32R, tag=f"adnT{k}", name=f"adnT{k}") for k in range(KI)]
            for k in range(KI):
                ps = ps_tr.tile([P, R], F32, tag="tr")
                nc.tensor.transpose(ps[:], adn_nat[:, P * k:P * (k + 1)],
                                    ident[:R, :R])
                nc.vector.tensor_copy(adnT[k][:], ps[:])
            # B_guT: [8 R, 4096 F]
            bgu_nat = small_pool.tile([P, F2 // P, R], F32, tag="bgu_nat")
            nc.sync.dma_start(bgu_nat[:],
                              bgu_d[:].rearrange("(bo bi) r -> bi bo r", bi=P))
            bguT = small_pool.tile([R, F2], F32R, tag="bguT")
            for bo in range(F2 // P):
                ps = ps_tr.tile([R, P], F32, tag="tr")
                nc.tensor.transpose(ps[:], bgu_nat[:, bo, :], ident[:])
                nc.vector.tensor_copy(bguT[:, P * bo:P * (bo + 1)], ps[:])
            # B_dnT: [8 R, 1024 H]
            bdn_nat = small_pool.tile([P, H // P, R], F32, tag="bdn_nat")
            nc.sync.dma_start(bdn_nat[:],
                              bdn_d[:].rearrange("(bo bi) r -> bi bo r", bi=P))
            bdnT = small_pool.tile([R, H], F32R, tag="bdnT")
            for bo in range(H // P):
                ps = ps_tr.tile([R, P], F32, tag="tr")
                nc.tensor.transpose(ps[:], bdn_nat[:, bo, :], ident[:])
                nc.vector.tensor_copy(bdnT[:, P * bo:P * (bo + 1)], ps[:])

            # ---- r1T = SCALING * (A_gu @ x^T): [8 R, 1024 T] ----
            r1T = small_pool.tile([R, T], F32R, tag="r1T")
            for t in range(NT):
                ps = ps_mm.tile([R, NFREE], F32, tag="mm")
                for k in range(KH):
                    nc.tensor.matmul(ps[:], aguT[k][:],
                                     xT[k][:, NFREE * t:NFREE * (t + 1)],
                                     start=(k == 0), stop=(k == KH - 1))
                nc.vector.tensor_scalar_mul(
                    r1T[:, NFREE * t:NFREE * (t + 1)], ps[:], SCALING)

            # ---- phase B: gate_up^T + SwiGLU -> hiddenT[k] [128 I, 1024 T] ----
            hT = [hT_pool.tile([P, T], F32R, tag=f"hT{k}", name=f"hT{k}") for k in range(KI)]
            for i in range(NFP):
                wg = wgu_pool.tile([P, KH, P], F32R, tag="wgu")
                wu = wgu_pool.tile([P, KH, P], F32R, tag="wgu")
                fg, fu = P * i, P * (i + NFP)
                nc.sync.dma_start(
                    wg[:], wgu_d[:, fg:fg + P].rearrange("(ko ki) f -> ki ko f", ki=P))
                nc.sync.dma_start(
                    wu[:], wgu_d[:, fu:fu + P].rearrange("(ko ki) f -> ki ko f", ki=P))
                for t in range(NT):
                    ts = slice(NFREE * t, NFREE * (t + 1))
                    psg = ps_mm.tile([P, NFREE], F32, tag="mm")
                    psu = ps_mm.tile([P, NFREE], F32, tag="mm")
                    for k in range(KH):
                        nc.tensor.matmul(psg[:], wg[:, k, :], xT[k][:, ts],
                                         start=(k == 0), stop=False)
                    nc.tensor.matmul(psg[:], bguT[:, fg:fg + P], r1T[:, ts],
                                     start=False, stop=True)
                    for k in range(KH):
                        nc.tensor.matmul(psu[:], wu[:, k, :], xT[k][:, ts],
                                         start=(k == 0), stop=False)
                    nc.tensor.matmul(psu[:], bguT[:, fu:fu + P], r1T[:, ts],
                                     start=False, stop=True)
                    sg = silu_pool.tile([P, NFREE], F32, tag="silu")
                    nc.scalar.activation(sg[:], psg[:],
                                         mybir.ActivationFunctionType.Silu)
                    nc.vector.tensor_mul(hT[i][:, ts], sg[:], psu[:])

            # ---- r2T = SCALING * (A_dn @ hidden^T): [8 R, 1024 T] ----
            r2T = small_pool.tile([R, T], F32R, tag="r2T")
            for t in range(NT):
                ps = ps_mm.tile([R, NFREE], F32, tag="mm")
                for k in range(KI):
                    nc.tensor.matmul(ps[:], adnT[k][:],
                                     hT[k][:, NFREE * t:NFREE * (t + 1)],
                                     start=(k == 0), stop=(k == KI - 1))
                nc.vector.tensor_scalar_mul(
                    r2T[:, NFREE * t:NFREE * (t + 1)], ps[:], SCALING)

            # ---- phase D: out[T, H] = hidden @ W_dn + lora ----
            for h in range(NH):
                hs = slice(NFREE * h, NFREE * (h + 1))
                for grp in range(2):
                    pos = []
                    for jj in range(4):
                        pos.append(ps_mm.tile([P, NFREE], F32, tag="mm", name=f"po{h}_{grp}_{jj}"))
                    for k in range(KI):
                        wd = wdn_pool.tile([P, NFREE], F32R, tag="wdn")
                        nc.sync.dma_start(wd[:], wdn_d[P * k:P * (k + 1), hs])
                        for jj in range(4):
                            j = 4 * grp + jj
                            nc.tensor.matmul(pos[jj][:],
                                             hT[k][:, P * j:P * (j + 1)], wd[:],
                                             start=(k == 0), stop=False)
                    for jj in range(4):
                        j = 4 * grp + jj
                        nc.tensor.matmul(pos[jj][:], r2T[:, P * j:P * (j + 1)],
                                         bdnT[:, hs], start=False, stop=True)
                        ot = out_pool.tile([P, NFREE], F32, tag="outs")
                        nc.scalar.activation(ot[:], pos[jj][:],
                                             mybir.ActivationFunctionType.Copy)
                        nc.sync.dma_start(out_d[P * j:P * (j + 1), hs], ot[:])

    nc.finalize()
    return nc


_NC_CACHE = None


def _get_nc():
    global _NC_CACHE
    if _NC_CACHE is None:
        _NC_CACHE = build_kernel()
    return _NC_CACHE


def _run(hidden_states, gate_up_proj, down_proj,
         lora_A_gu, lora_B_gu, lora_A_dn, lora_B_dn, **spmd_kwargs):
    f32 = np.float32
    hidden_states = np.ascontiguousarray(hidden_states, dtype=f32)
    gate_up_proj = np.ascontiguousarray(gate_up_proj, dtype=f32)
    down_proj = np.ascontiguousarray(down_proj, dtype=f32)
    lora_A_gu = np.ascontiguousarray(lora_A_gu, dtype=f32)
    lora_B_gu = np.ascontiguousarray(lora_B_gu, dtype=f32)
    lora_A_dn = np.ascontiguousarray(lora_A_dn, dtype=f32)
    lora_B_dn = np.ascontiguousarray(lora_B_dn, dtype=f32)

    nc = _get_nc()
    in_maps = []
    for e in range(E):
        in_maps.append({
            "x": hidden_states[T * e:T * (e + 1), :],
            "w_gu": gate_up_proj[e],
            "w_dn": down_proj[e],
            "a_gu": lora_A_gu,
            "b_gu": lora_B_gu,
            "a_dn": lora_A_dn,
            "b_dn": lora_B_dn,
        })
    res = run_bass_kernel_spmd(nc, in_maps, core_ids=list(range(E)),
                               **spmd_kwargs)
    out = np.concatenate([res.results[e]["out"] for e in range(E)], axis=0)
    return out, res


def kernel(hidden_states, gate_up_proj, down_proj,
           lora_A_gu, lora_B_gu, lora_A_dn, lora_B_dn):
    out, _ = _run(hidden_states, gate_up_proj, down_proj,
                  lora_A_gu, lora_B_gu, lora_A_dn, lora_B_dn)
    return out



# revision 2
# speedup vs baseline: 1.2430x; 1.2430x over previous
"""Llama4 MoE experts + shared LoRA, expert-parallel on 8 TRN2 NeuronCores.

Per-core (expert e): x[1024,1024] @ W_gu[1024,4096] (+ rank-8 LoRA) -> SwiGLU
-> h[1024,2048] @ W_dn[2048,1024] (+ rank-8 LoRA) -> out[1024,1024].

All matmul operands are bf16: on TRN2 hardware fp32/f32r matmuls run the
hi/lo-split path (~2 cycles/column observed); bf16 runs the native
1 cycle/column. Weights stream in as f32 and are cast to bf16 on the
otherwise-idle gpsimd engine; x is PE-transposed in f32 and cast on the
PSUM->SBUF copy; the SwiGLU multiply writes the intermediate directly as
bf16. PSUM accumulation stays f32, and the final output is stored f32.

W_dn is kept resident in SBUF as bf16 (32 KB/partition) so it is read from
HBM once (the f32r baseline read it twice).
"""
import sys

sys.path.insert(0, "/opt/trn_rl_repo")

import numpy as np

import concourse.bacc as bacc
import concourse.bass as bass
import concourse.mybir as mybir
import concourse.tile as tile
from concourse.bass_utils import run_bass_kernel_spmd
from concourse.masks import make_identity

E = 8           # experts == cores
T = 1024        # tokens per expert
H = 1024        # hidden
I = 2048        # intermediate
F2 = 2 * I      # gate+up
R = 8           # lora rank
SCALING = 2.0   # lora_alpha / rank
P = 128         # partitions
NFREE = 512     # moving free-dim per matmul (one PSUM bank of fp32)
KH = H // P     # 8 k-tiles over H
KI = I // P     # 16 k-tiles over I
NT = T // NFREE     # 2 T-chunks
NH = H // NFREE     # 2 H-chunks
NFP = I // P        # 16 F-pair tiles (gate i pairs with up i+16)

F32 = mybir.dt.float32
BF16 = mybir.dt.bfloat16


def build_kernel():
    nc = bacc.Bacc("TRN2", target_bir_lowering=False, debug=False)

    x_d = nc.dram_tensor("x", [T, H], F32, kind="ExternalInput")
    wgu_d = nc.dram_tensor("w_gu", [H, F2], F32, kind="ExternalInput")
    wdn_d = nc.dram_tensor("w_dn", [I, H], F32, kind="ExternalInput")
    agu_d = nc.dram_tensor("a_gu", [R, H], F32, kind="ExternalInput")
    bgu_d = nc.dram_tensor("b_gu", [F2, R], F32, kind="ExternalInput")
    adn_d = nc.dram_tensor("a_dn", [R, I], F32, kind="ExternalInput")
    bdn_d = nc.dram_tensor("b_dn", [H, R], F32, kind="ExternalInput")
    out_d = nc.dram_tensor("out", [T, H], F32, kind="ExternalOutput")

    with tile.TileContext(nc) as tc:
        with (
            tc.tile_pool(name="const", bufs=1) as const_pool,
            tc.tile_pool(name="xT", bufs=1) as xT_pool,
            tc.tile_pool(name="hT", bufs=1) as hT_pool,
            tc.tile_pool(name="wdnb", bufs=1) as wdnb_pool,
            tc.tile_pool(name="smalls", bufs=1) as small_pool,
            tc.tile_pool(name="xnat", bufs=2) as xnat_pool,
            tc.tile_pool(name="wgu", bufs=4) as wgu_pool,
            tc.tile_pool(name="wgub", bufs=4) as wgub_pool,
            tc.tile_pool(name="wdn", bufs=3) as wdn_pool,
            tc.tile_pool(name="silu", bufs=3) as silu_pool,
            tc.tile_pool(name="outs", bufs=3) as out_pool,
            tc.tile_pool(name="ps_tr", bufs=2, space="PSUM") as ps_tr,
            tc.tile_pool(name="ps_mm", bufs=6, space="PSUM") as ps_mm,
        ):
            ident = const_pool.tile([P, P], F32)
            make_identity(nc, ident[:])

            # ---- phase A: transpose x into xT[k] = [128 H, 1024 T] bf16 ----
            xT = [xT_pool.tile([P, T], BF16, tag=f"xT{k}", name=f"xT{k}") for k in range(KH)]
            for b in range(T // P):
                xb = xnat_pool.tile([P, H], F32)
                nc.sync.dma_start(xb[:], x_d[P * b:P * (b + 1), :])
                for k in range(KH):
                    ps = ps_tr.tile([P, P], F32, tag="tr")
                    nc.tensor.transpose(ps[:], xb[:, P * k:P * (k + 1)], ident[:])
                    nc.vector.tensor_copy(xT[k][:, P * b:P * (b + 1)], ps[:])

            # ---- small lora tensors: transpose to matmul layouts (bf16) ----
            # A_guT[k]: [128 H, 8 R]
            agu_nat = small_pool.tile([R, H], F32, tag="agu_nat")
            nc.sync.dma_start(agu_nat[:], agu_d[:])
            aguT = [small_pool.tile([P, R], BF16, tag=f"aguT{k}", name=f"aguT{k}") for k in range(KH)]
            for k in range(KH):
                ps = ps_tr.tile([P, R], F32, tag="tr")
                nc.tensor.transpose(ps[:], agu_nat[:, P * k:P * (k + 1)],
                                    ident[:R, :R])
                nc.vector.tensor_copy(aguT[k][:], ps[:])
            # A_dnT[k]: [128 I, 8 R]
            adn_nat = small_pool.tile([R, I], F32, tag="adn_nat")
            nc.sync.dma_start(adn_nat[:], adn_d[:])
            adnT = [small_pool.tile([P, R], BF16, tag=f"adnT{k}", name=f"adnT{k}") for k in range(KI)]
            for k in range(KI):
                ps = ps_tr.tile([P, R], F32, tag="tr")
                nc.tensor.transpose(ps[:], adn_nat[:, P * k:P * (k + 1)],
                                    ident[:R, :R])
                nc.vector.tensor_copy(adnT[k][:], ps[:])
            # B_guT: [8 R, 4096 F]
            bgu_nat = small_pool.tile([P, F2 // P, R], F32, tag="bgu_nat")
            nc.sync.dma_start(bgu_nat[:],
                              bgu_d[:].rearrange("(bo bi) r -> bi bo r", bi=P))
            bguT = small_pool.tile([R, F2], BF16, tag="bguT")
            for bo in range(F2 // P):
                ps = ps_tr.tile([R, P], F32, tag="tr")
                nc.tensor.transpose(ps[:], bgu_nat[:, bo, :], ident[:])
                nc.vector.tensor_copy(bguT[:, P * bo:P * (bo + 1)], ps[:])
            # B_dnT: [8 R, 1024 H]
            bdn_nat = small_pool.tile([P, H // P, R], F32, tag="bdn_nat")
            nc.sync.dma_start(bdn_nat[:],
                              bdn_d[:].rearrange("(bo bi) r -> bi bo r", bi=P))
            bdnT = small_pool.tile([R, H], BF16, tag="bdnT")
            for bo in range(H // P):
                ps = ps_tr.tile([R, P], F32, tag="tr")
                nc.tensor.transpose(ps[:], bdn_nat[:, bo, :], ident[:])
                nc.vector.tensor_copy(bdnT[:, P * bo:P * (bo + 1)], ps[:])

            # ---- r1T = SCALING * (A_gu @ x^T): [8 R, 1024 T] bf16 ----
            r1T = small_pool.tile([R, T], BF16, tag="r1T")
            for t in range(NT):
                ps = ps_mm.tile([R, NFREE], F32, tag="mm")
                for k in range(KH):
                    nc.tensor.matmul(ps[:], aguT[k][:],
                                     xT[k][:, NFREE * t:NFREE * (t + 1)],
                                     start=(k == 0), stop=(k == KH - 1))
                nc.vector.tensor_scalar_mul(
                    r1T[:, NFREE * t:NFREE * (t + 1)], ps[:], SCALING)

            # ---- phase B: gate_up^T + SwiGLU -> hiddenT[k] [128 I, 1024 T] ----
            hT = [hT_pool.tile([P, T], BF16, tag=f"hT{k}", name=f"hT{k}") for k in range(KI)]
            for i in range(NFP):
                wg = wgu_pool.tile([P, KH, P], F32, tag="wgu")
                wu = wgu_pool.tile([P, KH, P], F32, tag="wgu")
                fg, fu = P * i, P * (i + NFP)
                nc.sync.dma_start(
                    wg[:], wgu_d[:, fg:fg + P].rearrange("(ko ki) f -> ki ko f", ki=P))
                nc.sync.dma_start(
                    wu[:], wgu_d[:, fu:fu + P].rearrange("(ko ki) f -> ki ko f", ki=P))
                wgb = wgub_pool.tile([P, KH, P], BF16, tag="wgub")
                wub = wgub_pool.tile([P, KH, P], BF16, tag="wgub")
                nc.gpsimd.tensor_copy(wgb[:], wg[:])
                nc.gpsimd.tensor_copy(wub[:], wu[:])
                for t in range(NT):
                    ts = slice(NFREE * t, NFREE * (t + 1))
                    psg = ps_mm.tile([P, NFREE], F32, tag="mm")
                    psu = ps_mm.tile([P, NFREE], F32, tag="mm")
                    for k in range(KH):
                        nc.tensor.matmul(psg[:], wgb[:, k, :], xT[k][:, ts],
                                         start=(k == 0), stop=False)
                    nc.tensor.matmul(psg[:], bguT[:, fg:fg + P], r1T[:, ts],
                                     start=False, stop=True)
                    for k in range(KH):
                        nc.tensor.matmul(psu[:], wub[:, k, :], xT[k][:, ts],
                                         start=(k == 0), stop=False)
                    nc.tensor.matmul(psu[:], bguT[:, fu:fu + P], r1T[:, ts],
                                     start=False, stop=True)
                    sg = silu_pool.tile([P, NFREE], F32, tag="silu")
                    nc.scalar.activation(sg[:], psg[:],
                                         mybir.ActivationFunctionType.Silu)
                    nc.vector.tensor_mul(hT[i][:, ts], sg[:], psu[:])

            # ---- r2T = SCALING * (A_dn @ hidden^T): [8 R, 1024 T] bf16 ----
            r2T = small_pool.tile([R, T], BF16, tag="r2T")
            for t in range(NT):
                ps = ps_mm.tile([R, NFREE], F32, tag="mm")
                for k in range(KI):
                    nc.tensor.matmul(ps[:], adnT[k][:],
                                     hT[k][:, NFREE * t:NFREE * (t + 1)],
                                     start=(k == 0), stop=(k == KI - 1))
                nc.vector.tensor_scalar_mul(
                    r2T[:, NFREE * t:NFREE * (t + 1)], ps[:], SCALING)

            # ---- phase D: out[T, H] = hidden @ W_dn + lora ----
            # wdnb[k] holds full W_dn rows [128 I, 1024 H] bf16, resident for
            # the whole phase: HBM-read once, used by all four (h, grp) passes.
            wdnb = [wdnb_pool.tile([P, H], BF16, tag=f"wdnb{k}", name=f"wdnb{k}")
                    for k in range(KI)]
            first = True
            for h in range(NH):
                hs = slice(NFREE * h, NFREE * (h + 1))
                for grp in range(2):
                    pos = []
                    for jj in range(4):
                        pos.append(ps_mm.tile([P, NFREE], F32, tag="mm", name=f"po{h}_{grp}_{jj}"))
                    for k in range(KI):
                        if first:
                            wd = wdn_pool.tile([P, H], F32, tag="wdn")
                            nc.sync.dma_start(wd[:], wdn_d[P * k:P * (k + 1), :])
                            nc.gpsimd.tensor_copy(wdnb[k][:], wd[:])
                        for jj in range(4):
                            j = 4 * grp + jj
                            nc.tensor.matmul(pos[jj][:],
                                             hT[k][:, P * j:P * (j + 1)],
                                             wdnb[k][:, hs],
                                             start=(k == 0), stop=False)
                    first = False
                    for jj in range(4):
                        j = 4 * grp + jj
                        nc.tensor.matmul(pos[jj][:], r2T[:, P * j:P * (j + 1)],
                                         bdnT[:, hs], start=False, stop=True)
                        ot = out_pool.tile([P, NFREE], F32, tag="outs")
                        nc.scalar.activation(ot[:], pos[jj][:],
                                             mybir.ActivationFunctionType.Copy)
                        nc.sync.dma_start(out_d[P * j:P * (j + 1), hs], ot[:])

    nc.finalize()
    return nc


_NC_CACHE = None


def _get_nc():
    global _NC_CACHE
    if _NC_CACHE is None:
        _NC_CACHE = build_kernel()
    return _NC_CACHE


def _run(hidden_states, gate_up_proj, down_proj,
         lora_A_gu, lora_B_gu, lora_A_dn, lora_B_dn, **spmd_kwargs):
    f32 = np.float32
    hidden_states = np.ascontiguousarray(hidden_states, dtype=f32)
    gate_up_proj = np.ascontiguousarray(gate_up_proj, dtype=f32)
    down_proj = np.ascontiguousarray(down_proj, dtype=f32)
    lora_A_gu = np.ascontiguousarray(lora_A_gu, dtype=f32)
    lora_B_gu = np.ascontiguousarray(lora_B_gu, dtype=f32)
    lora_A_dn = np.ascontiguousarray(lora_A_dn, dtype=f32)
    lora_B_dn = np.ascontiguousarray(lora_B_dn, dtype=f32)

    nc = _get_nc()
    in_maps = []
    for e in range(E):
        in_maps.append({
            "x": hidden_states[T * e:T * (e + 1), :],
            "w_gu": gate_up_proj[e],
            "w_dn": down_proj[e],
            "a_gu": lora_A_gu,
            "b_gu": lora_B_gu,
            "a_dn": lora_A_dn,
            "b_dn": lora_B_dn,
        })
    res = run_bass_kernel_spmd(nc, in_maps, core_ids=list(range(E)),
                               **spmd_kwargs)
    out = np.concatenate([res.results[e]["out"] for e in range(E)], axis=0)
    return out, res


def kernel(hidden_states, gate_up_proj, down_proj,
           lora_A_gu, lora_B_gu, lora_A_dn, lora_B_dn):
    out, _ = _run(hidden_states, gate_up_proj, down_proj,
                  lora_A_gu, lora_B_gu, lora_A_dn, lora_B_dn)
    return out


# revision 3
# speedup vs baseline: 1.2929x; 1.0401x over previous
"""Llama4 MoE experts + shared LoRA, expert-parallel on 8 TRN2 NeuronCores.

Per-core (expert e): x[1024,1024] @ W_gu[1024,4096] (+ rank-8 LoRA) -> SwiGLU
-> h[1024,2048] @ W_dn[2048,1024] (+ rank-8 LoRA) -> out[1024,1024].

All matmul operands are bf16 (native 1 cycle/column on TRN2; fp32/f32r pay a
hi/lo split). Weights stream in as f32 and are cast to bf16 on the vector /
scalar / gpsimd engines round-robin so no single engine starves the PE. x is
cast to bf16 and transposed by the DMA XBAR (dma transpose), not the PE.

Matmuls that share a stationary operand are issued back-to-back (both T-chunks
per W_gu tile, both H-halves per hT chunk) so the fixed ~173ns PE SBUF access
latency and the LDWEIGHTS amortize over 2x512 moving columns.

W_dn is kept resident in SBUF as bf16 (32 KB/partition): HBM-read once.
PSUM accumulates in f32; output is stored f32.
"""
import sys

sys.path.insert(0, "/opt/trn_rl_repo")

import numpy as np

import concourse.bacc as bacc
import concourse.bass as bass
import concourse.mybir as mybir
import concourse.tile as tile
from concourse.bass_utils import run_bass_kernel_spmd
from concourse.masks import make_identity

E = 8           # experts == cores
T = 1024        # tokens per expert
H = 1024        # hidden
I = 2048        # intermediate
F2 = 2 * I      # gate+up
R = 8           # lora rank
SCALING = 2.0   # lora_alpha / rank
P = 128         # partitions
NFREE = 512     # moving free-dim per matmul (one PSUM bank of fp32)
KH = H // P     # 8 k-tiles over H
KI = I // P     # 16 k-tiles over I
NT = T // NFREE     # 2 T-chunks
NH = H // NFREE     # 2 H-chunks
NFP = I // P        # 16 F-pair tiles (gate i pairs with up i+16)

F32 = mybir.dt.float32
BF16 = mybir.dt.bfloat16


def build_kernel():
    nc = bacc.Bacc("TRN2", target_bir_lowering=False, debug=False)

    x_d = nc.dram_tensor("x", [T, H], F32, kind="ExternalInput")
    wgu_d = nc.dram_tensor("w_gu", [H, F2], F32, kind="ExternalInput")
    wdn_d = nc.dram_tensor("w_dn", [I, H], F32, kind="ExternalInput")
    agu_d = nc.dram_tensor("a_gu", [R, H], F32, kind="ExternalInput")
    bgu_d = nc.dram_tensor("b_gu", [F2, R], F32, kind="ExternalInput")
    adn_d = nc.dram_tensor("a_dn", [R, I], F32, kind="ExternalInput")
    bdn_d = nc.dram_tensor("b_dn", [H, R], F32, kind="ExternalInput")
    out_d = nc.dram_tensor("out", [T, H], F32, kind="ExternalOutput")

    with tile.TileContext(nc) as tc:
        with (
            tc.tile_pool(name="const", bufs=1) as const_pool,
            tc.tile_pool(name="xT", bufs=1) as xT_pool,
            tc.tile_pool(name="hT", bufs=1) as hT_pool,
            tc.tile_pool(name="wdnb", bufs=1) as wdnb_pool,
            tc.tile_pool(name="smalls", bufs=1) as small_pool,
            tc.tile_pool(name="xnat", bufs=2) as xnat_pool,
            tc.tile_pool(name="xbf", bufs=2) as xbf_pool,
            tc.tile_pool(name="wgu", bufs=4) as wgu_pool,
            tc.tile_pool(name="wgub", bufs=4) as wgub_pool,
            tc.tile_pool(name="wdn", bufs=3) as wdn_pool,
            tc.tile_pool(name="silu", bufs=4) as silu_pool,
            tc.tile_pool(name="outs", bufs=3) as out_pool,
            tc.tile_pool(name="ps_tr", bufs=2, space="PSUM") as ps_tr,
            tc.tile_pool(name="ps_mm", bufs=6, space="PSUM") as ps_mm,
        ):
            ident = const_pool.tile([P, P], F32)
            make_identity(nc, ident[:])

            # ---- phase A: x -> bf16 -> XBAR transpose -> xT[p, k, t] ----
            # xT[:, k, t] = x[t, 128k + p]: standard k-tile layout, produced
            # entirely by DMA-transpose; the PE never touches x.
            xT = xT_pool.tile([P, KH, T], BF16, tag="xT", name="xT")
            for b in range(T // P):
                xb = xnat_pool.tile([P, H], F32)
                nc.sync.dma_start(xb[:], x_d[P * b:P * (b + 1), :])
                xbf = xbf_pool.tile([P, H], BF16)
                nc.vector.tensor_copy(xbf[:], xb[:])
                nc.sync.dma_start(xT[:, :, P * b:P * (b + 1)], xbf[:],
                                  transpose=True)

            # ---- small lora tensors: transpose to matmul layouts (bf16) ----
            # A_guT[k]: [128 H, 8 R]
            agu_nat = small_pool.tile([R, H], F32, tag="agu_nat")
            nc.sync.dma_start(agu_nat[:], agu_d[:])
            aguT = [small_pool.tile([P, R], BF16, tag=f"aguT{k}", name=f"aguT{k}") for k in range(KH)]
            for k in range(KH):
                ps = ps_tr.tile([P, R], F32, tag="tr")
                nc.tensor.transpose(ps[:], agu_nat[:, P * k:P * (k + 1)],
                                    ident[:R, :R])
                nc.vector.tensor_copy(aguT[k][:], ps[:])
            # A_dnT[k]: [128 I, 8 R]
            adn_nat = small_pool.tile([R, I], F32, tag="adn_nat")
            nc.sync.dma_start(adn_nat[:], adn_d[:])
            adnT = [small_pool.tile([P, R], BF16, tag=f"adnT{k}", name=f"adnT{k}") for k in range(KI)]
            for k in range(KI):
                ps = ps_tr.tile([P, R], F32, tag="tr")
                nc.tensor.transpose(ps[:], adn_nat[:, P * k:P * (k + 1)],
                                    ident[:R, :R])
                nc.vector.tensor_copy(adnT[k][:], ps[:])
            # B_guT: [8 R, 4096 F]
            bgu_nat = small_pool.tile([P, F2 // P, R], F32, tag="bgu_nat")
            nc.sync.dma_start(bgu_nat[:],
                              bgu_d[:].rearrange("(bo bi) r -> bi bo r", bi=P))
            bguT = small_pool.tile([R, F2], BF16, tag="bguT")
            for bo in range(F2 // P):
                ps = ps_tr.tile([R, P], F32, tag="tr")
                nc.tensor.transpose(ps[:], bgu_nat[:, bo, :], ident[:])
                nc.vector.tensor_copy(bguT[:, P * bo:P * (bo + 1)], ps[:])
            # B_dnT: [8 R, 1024 H]
            bdn_nat = small_pool.tile([P, H // P, R], F32, tag="bdn_nat")
            nc.sync.dma_start(bdn_nat[:],
                              bdn_d[:].rearrange("(bo bi) r -> bi bo r", bi=P))
            bdnT = small_pool.tile([R, H], BF16, tag="bdnT")
            for bo in range(H // P):
                ps = ps_tr.tile([R, P], F32, tag="tr")
                nc.tensor.transpose(ps[:], bdn_nat[:, bo, :], ident[:])
                nc.vector.tensor_copy(bdnT[:, P * bo:P * (bo + 1)], ps[:])

            # ---- r1T = SCALING * (A_gu @ x^T): [8 R, 1024 T] bf16 ----
            r1T = small_pool.tile([R, T], BF16, tag="r1T")
            rps = [ps_mm.tile([R, NFREE], F32, tag="mm", name=f"r1ps{t}")
                   for t in range(NT)]
            for k in range(KH):
                for t in range(NT):
                    nc.tensor.matmul(rps[t][:], aguT[k][:],
                                     xT[:, k, NFREE * t:NFREE * (t + 1)],
                                     start=(k == 0), stop=(k == KH - 1))
            for t in range(NT):
                nc.vector.tensor_scalar_mul(
                    r1T[:, NFREE * t:NFREE * (t + 1)], rps[t][:], SCALING)

            # ---- phase B: gate_up^T + SwiGLU -> hiddenT[k] [128 I, 1024 T] ----
            hT = [hT_pool.tile([P, T], BF16, tag=f"hT{k}", name=f"hT{k}") for k in range(KI)]
            for i in range(NFP):
                wg = wgu_pool.tile([P, KH, P], F32, tag="wgu")
                wu = wgu_pool.tile([P, KH, P], F32, tag="wgu")
                fg, fu = P * i, P * (i + NFP)
                nc.sync.dma_start(
                    wg[:], wgu_d[:, fg:fg + P].rearrange("(ko ki) f -> ki ko f", ki=P))
                nc.sync.dma_start(
                    wu[:], wgu_d[:, fu:fu + P].rearrange("(ko ki) f -> ki ko f", ki=P))
                wgb = wgub_pool.tile([P, KH, P], BF16, tag="wgub")
                wub = wgub_pool.tile([P, KH, P], BF16, tag="wgub")
                nc.vector.tensor_copy(wgb[:], wg[:])
                nc.gpsimd.tensor_copy(wub[:], wu[:])
                psg = [ps_mm.tile([P, NFREE], F32, tag="mm", name=f"psg{i}_{t}")
                       for t in range(NT)]
                psu = [ps_mm.tile([P, NFREE], F32, tag="mm", name=f"psu{i}_{t}")
                       for t in range(NT)]
                for k in range(KH):
                    for t in range(NT):
                        ts = slice(NFREE * t, NFREE * (t + 1))
                        nc.tensor.matmul(psg[t][:], wgb[:, k, :], xT[:, k, ts],
                                         start=(k == 0), stop=False)
                for t in range(NT):
                    ts = slice(NFREE * t, NFREE * (t + 1))
                    nc.tensor.matmul(psg[t][:], bguT[:, fg:fg + P], r1T[:, ts],
                                     start=False, stop=True)
                for k in range(KH):
                    for t in range(NT):
                        ts = slice(NFREE * t, NFREE * (t + 1))
                        nc.tensor.matmul(psu[t][:], wub[:, k, :], xT[:, k, ts],
                                         start=(k == 0), stop=False)
                for t in range(NT):
                    ts = slice(NFREE * t, NFREE * (t + 1))
                    nc.tensor.matmul(psu[t][:], bguT[:, fu:fu + P], r1T[:, ts],
                                     start=False, stop=True)
                for t in range(NT):
                    ts = slice(NFREE * t, NFREE * (t + 1))
                    sg = silu_pool.tile([P, NFREE], F32, tag="silu")
                    nc.scalar.activation(sg[:], psg[t][:],
                                         mybir.ActivationFunctionType.Silu)
                    nc.vector.tensor_mul(hT[i][:, ts], sg[:], psu[t][:])

            # ---- r2T = SCALING * (A_dn @ hidden^T): [8 R, 1024 T] bf16 ----
            r2T = small_pool.tile([R, T], BF16, tag="r2T")
            rps2 = [ps_mm.tile([R, NFREE], F32, tag="mm", name=f"r2ps{t}")
                    for t in range(NT)]
            for k in range(KI):
                for t in range(NT):
                    nc.tensor.matmul(rps2[t][:], adnT[k][:],
                                     hT[k][:, NFREE * t:NFREE * (t + 1)],
                                     start=(k == 0), stop=(k == KI - 1))
            for t in range(NT):
                nc.vector.tensor_scalar_mul(
                    r2T[:, NFREE * t:NFREE * (t + 1)], rps2[t][:], SCALING)

            # ---- phase D: out[T, H] = hidden @ W_dn + lora ----
            # wdnb[k] holds full W_dn rows [128 I, 1024 H] bf16, resident for
            # the whole phase: HBM-read once, reused by all four j-pair passes.
            # Per stationary hT chunk, both H-halves are issued back-to-back.
            wdnb = [wdnb_pool.tile([P, H], BF16, tag=f"wdnb{k}", name=f"wdnb{k}")
                    for k in range(KI)]
            cast_engines = [nc.vector, nc.scalar, nc.gpsimd]
            first = True
            for grp in range(4):
                pos = [[ps_mm.tile([P, NFREE], F32, tag="mm",
                                   name=f"po{grp}_{jj}_{h}")
                        for h in range(NH)] for jj in range(2)]
                for k in range(KI):
                    if first:
                        wd = wdn_pool.tile([P, H], F32, tag="wdn")
                        nc.sync.dma_start(wd[:], wdn_d[P * k:P * (k + 1), :])
                        eng = cast_engines[k % 3]
                        if eng is nc.scalar:
                            eng.copy(wdnb[k][:], wd[:])
                        else:
                            eng.tensor_copy(wdnb[k][:], wd[:])
                    for jj in range(2):
                        j = 2 * grp + jj
                        for h in range(NH):
                            hs = slice(NFREE * h, NFREE * (h + 1))
                            nc.tensor.matmul(pos[jj][h][:],
                                             hT[k][:, P * j:P * (j + 1)],
                                             wdnb[k][:, hs],
                                             start=(k == 0), stop=False)
                first = False
                for jj in range(2):
                    j = 2 * grp + jj
                    for h in range(NH):
                        hs = slice(NFREE * h, NFREE * (h + 1))
                        nc.tensor.matmul(pos[jj][h][:], r2T[:, P * j:P * (j + 1)],
                                         bdnT[:, hs], start=False, stop=True)
                        ot = out_pool.tile([P, NFREE], F32, tag="outs")
                        nc.scalar.activation(ot[:], pos[jj][h][:],
                                             mybir.ActivationFunctionType.Copy)
                        nc.sync.dma_start(out_d[P * j:P * (j + 1), hs], ot[:])

    nc.finalize()
    return nc


_NC_CACHE = None


def _get_nc():
    global _NC_CACHE
    if _NC_CACHE is None:
        _NC_CACHE = build_kernel()
    return _NC_CACHE


def _run(hidden_states, gate_up_proj, down_proj,
         lora_A_gu, lora_B_gu, lora_A_dn, lora_B_dn, **spmd_kwargs):
    f32 = np.float32
    hidden_states = np.ascontiguousarray(hidden_states, dtype=f32)
    gate_up_proj = np.ascontiguousarray(gate_up_proj, dtype=f32)
    down_proj = np.ascontiguousarray(down_proj, dtype=f32)
    lora_A_gu = np.ascontiguousarray(lora_A_gu, dtype=f32)
    lora_B_gu = np.ascontiguousarray(lora_B_gu, dtype=f32)
    lora_A_dn = np.ascontiguousarray(lora_A_dn, dtype=f32)
    lora_B_dn = np.ascontiguousarray(lora_B_dn, dtype=f32)

    nc = _get_nc()
    in_maps = []
    for e in range(E):
        in_maps.append({
            "x": hidden_states[T * e:T * (e + 1), :],
            "w_gu": gate_up_proj[e],
            "w_dn": down_proj[e],
            "a_gu": lora_A_gu,
            "b_gu": lora_B_gu,
            "a_dn": lora_A_dn,
            "b_dn": lora_B_dn,
        })
    res = run_bass_kernel_spmd(nc, in_maps, core_ids=list(range(E)),
                               **spmd_kwargs)
    out = np.concatenate([res.results[e]["out"] for e in range(E)], axis=0)
    return out, res


def kernel(hidden_states, gate_up_proj, down_proj,
           lora_A_gu, lora_B_gu, lora_A_dn, lora_B_dn):
    out, _ = _run(hidden_states, gate_up_proj, down_proj,
                  lora_A_gu, lora_B_gu, lora_A_dn, lora_B_dn)
    return out
